# revision 19
# baseline (speedup 1.0000x reference)
"""Cross-attention kernel for 8 TRN2 NeuronCores (Bass/Tile).

Reference computation (fp32):
    q = x @ Wq; k = ctx @ Wk; v = ctx @ Wv        (reshape to heads, d=64)
    sim = q k^T * d^-0.5 ; attn = softmax(sim)
    out = (attn v) @ Wo + bo

Shapes: x [4, 2048, 1024], context [4, 1024, 768], 8 heads * 64, out [4, 2048, 1024].

Sharding (data ||): core c -> batch b=c//2, head-group hg=c%2 (4 heads).
Each core computes a partial output [2048, 1024] = attn-out(4 heads) @ Wo[hg-slice];
host sums the two partials per batch and adds bo.  No FLOP duplication
(5.1 GFLOP/core = total/8).

Per-core layout strategy (bf16 matmul operands, fp32 accumulation):
  - host passes x^T [1024f, 2048i] and ctx^T [768f, 1024j]  (contraction dims on
    partitions; avoids any on-device transposes)
  - QT[d,i] = matmul(lhsT=Wq, rhs=xT);  KT[d,j] = matmul(lhsT=Wk, rhs=ctxT)
  - V[j,d]  = matmul(lhsT=ctxT, rhs=Wv), stored per-head as [V_h | 1] (ones col)
  - simT[j,i] = matmul(lhsT=KT_h, rhs=QT_h)        (K=64 contraction)
  - expT = exp(0.125*simT)  (ScalarE, unsafe softmax: |sim|*scale < ~4, exact)
  - av' = matmul(lhsT=[V_h|1], rhs=expT) accumulated over j -> [65, i]:
        rows 0..63 = unnormalised attn-out^T, row 64 = softmax denominator
  - recip(denom) broadcast j->partitions via DRAM round-trip DMA;
    out2T = av'[0:64] * recip  (DVE)  -> exactly the lhsT layout Wo needs
  - out[i,e] = matmul(lhsT=out2T, rhs=Wo)  -> DMA to DRAM (f32 partial)
"""

import numpy as np
import ml_dtypes

import concourse.bass as bass
import concourse.tile as tile
from concourse import bacc, mybir
from concourse.bass_utils import run_bass_kernel_spmd

# problem constants (hardcoded per the harness contract)
B = 4              # batches
I = 2048           # query seq len
J = 1024           # context seq len
FQ = 1024          # query dim
FC = 768           # context dim
DH = 64            # head dim
HPC = 4            # heads per core
DG = HPC * DH      # 256: per-core slice of inner dim
E = 1024           # output dim
P = 128
N_CORES = 8
IH = I // 2        # 1024: i-half processed per attention pass

F32 = mybir.dt.float32
BF16 = mybir.dt.bfloat16

KQ = FQ // P       # 8  k-blocks for q projection
KC = FC // P       # 6  k-blocks for k/v projection
TD = DG // P       # 2  partition-blocks of the per-core inner dim
JBN = J // P       # 8  j-blocks


def _build():
    nc = bacc.Bacc()
    # inputs arrive pre-shuffled into their exact SBUF images:
    # [128 partitions, <k-blocks> * <row>] so every partition's data is one
    # fat contiguous DRAM run (max DMA packet efficiency)
    xt = nc.declare_dram_parameter("xt", [P, KQ * I], BF16, isOutput=False)
    ctxt = nc.declare_dram_parameter("ctxt", [P, KC * J], BF16, isOutput=False)
    wq = nc.declare_dram_parameter("wq", [P, KQ * DG], BF16, isOutput=False)
    wk = nc.declare_dram_parameter("wk", [P, KC * DG], BF16, isOutput=False)
    wv = nc.declare_dram_parameter("wv", [P, KC * DG], BF16, isOutput=False)
    wo = nc.declare_dram_parameter("wo", [P, TD * E], BF16, isOutput=False)
    vones = nc.declare_dram_parameter("vones", [P, HPC], BF16, isOutput=False)
    out = nc.declare_dram_parameter("out", [I, E], BF16, isOutput=True)
    brc = nc.dram_tensor("brc", [2 * HPC, IH], F32)   # denom rows (j->dram)
    brc2 = nc.dram_tensor("brc2", [2 * HPC, IH], F32)  # recip rows for bcast

    with tile.TileContext(nc) as tc:
        with (
            tc.tile_pool(name="consts", bufs=1) as consts,
            tc.tile_pool(name="expp", bufs=26) as expp,
            tc.tile_pool(name="misc", bufs=3) as misc,
            tc.tile_pool(name="outp", bufs=3) as outp,
            tc.tile_pool(name="pp", bufs=2, space="PSUM") as pp,
            tc.tile_pool(name="avp", bufs=2, space="PSUM") as avpool,
        ):
            # ---------------- persistent loads ----------------
            # PE warm-up: ~9us of junk matmuls from t~0 so HAM un-throttles
            # (cold PE runs at 1.2 GHz; the free-running 3.4us activity
            # window needs sustained work) and projections start warm
            junk = consts.tile([P, 512], BF16, tag="junk")
            nc.vector.memset(junk, 0.0)
            jps = pp.tile([P, IH], F32, tag="pp", name="jps")
            for w in range(36):
                nc.tensor.matmul(jps[:, :P], lhsT=junk[:, :P],
                                 rhs=junk[:, :P], start=True, stop=True)

            # all loads on the sync HWDGE queue, ordered by first use;
            # ctx and x are split per k-block so the accumulating matmuls
            # can start as soon as each block lands
            wk_sb = consts.tile([P, KC, DG], BF16, tag="wk_sb")
            nc.sync.dma_start(
                out=wk_sb, in_=wk[:, :].rearrange("p (kb d) -> p kb d", kb=KC))
            wv_sb = consts.tile([P, KC, DG], BF16, tag="wv_sb")
            nc.sync.dma_start(
                out=wv_sb, in_=wv[:, :].rearrange("p (kb d) -> p kb d", kb=KC))
            wq_sb = consts.tile([P, KQ, DG], BF16, tag="wq_sb")
            nc.sync.dma_start(
                out=wq_sb, in_=wq[:, :].rearrange("p (kb d) -> p kb d", kb=KQ))
            ctxt_sb = consts.tile([P, KC, J], BF16, tag="ctxt_sb")
            nc.sync.dma_start(
                out=ctxt_sb,
                in_=ctxt[:, :].rearrange("p (kb j) -> p kb j", kb=KC))

            # ---------------- projections ----------------
            # KT [d=256, j=1024]: lhsT = wk [f, d], rhs = ctxT [f, j]
            kt_sb = [consts.tile([P, J], BF16, tag=f"kt{t}", name=f"kt{t}") for t in range(TD)]
            for t in range(TD):
                ps = pp.tile([P, IH], F32, tag="pp")
                for kb in range(KC):
                    for nchunk in range(2):
                        nc.tensor.matmul(
                            ps[:, nchunk * 512:(nchunk + 1) * 512],
                            lhsT=wk_sb[:, kb, t * P:(t + 1) * P],
                            rhs=ctxt_sb[:, kb, nchunk * 512:(nchunk + 1) * 512],
                            start=(kb == 0), stop=(kb == KC - 1),
                        )
                nc.vector.tensor_copy(kt_sb[t], ps)

            # V [j, d] per-head with ones column: v_sb[jb] = [128, HPC, 65]
            v_sb = [consts.tile([P, HPC, DH + 1], BF16, tag=f"v{jb}", name=f"v{jb}")
                    for jb in range(JBN)]
            for jb in range(JBN):
                nc.gpsimd.dma_start(
                    out=v_sb[jb][:, :, DH:DH + 1],
                    in_=vones[:, :].rearrange("p (h o) -> p h o", o=1),
                )
                ps = avpool.tile([P, DG], F32, tag="av", name="vps")
                for kb in range(KC):
                    nc.tensor.matmul(
                        ps,
                        lhsT=ctxt_sb[:, kb, jb * P:(jb + 1) * P],
                        rhs=wv_sb[:, kb, :],
                        start=(kb == 0), stop=(kb == KC - 1),
                    )
                nc.vector.tensor_copy(
                    v_sb[jb][:, :, 0:DH],
                    ps.rearrange("p (h d) -> p h d", h=HPC),
                )

            # QT [d=256, i=2048]: lhsT = wq [f, d], rhs = xT [f, i]
            qt_sb = [consts.tile([P, I], BF16, tag=f"qt{t}", name=f"qt{t}") for t in range(TD)]
            xq_sb = consts.tile([P, KQ, I], BF16, tag="xq_sb")
            nc.sync.dma_start(
                out=xq_sb,
                in_=xt[:, :].rearrange("p (kb i) -> p kb i", kb=KQ))
            wo_sb = consts.tile([P, TD, E], BF16, tag="wo_sb")
            nc.sync.dma_start(
                out=wo_sb, in_=wo[:, :].rearrange("p (kb e) -> p kb e", kb=TD))
            def emit_qt(ich, t):
                isl = slice(ich * 512, (ich + 1) * 512)
                ps = pp.tile([P, IH], F32, tag="pp", name="ps")
                for kb in range(KQ):
                    nc.tensor.matmul(
                        ps[:, :512],
                        lhsT=wq_sb[:, kb, t * P:(t + 1) * P],
                        rhs=xq_sb[:, kb, isl],
                        start=(kb == 0), stop=(kb == KQ - 1),
                    )
                nc.vector.tensor_copy(qt_sb[t][:, isl], ps[:, :512])

            emit_qt(0, 0)
            emit_qt(1, 0)

            # ---------------- attention + output projection ----------------
            o2t_sb = [[consts.tile([P, IH], BF16, tag=f"o2t{half}{t}", name=f"o2t{half}{t}")
                       for t in range(TD)] for half in range(2)]

            # attention: software-pipelined head-pairs so ScalarE (exp)
            # never waits on a pair transition. Pair p+1's scores/exp are
            # emitted before pair p's AV matmuls + normalisation.
            def emit_scores_exp(half, hp):
                t = hp
                ets = [[None] * JBN, [None] * JBN]
                for jb in range(JBN):
                    scs = []
                    for par in range(2):
                        prow = par * DH
                        sc = pp.tile([P, IH], F32, tag="pp", name=f"sc{par}")
                        for nchunk in range(2):
                            csl = slice(nchunk * 512, (nchunk + 1) * 512)
                            qsl = slice(half * IH + nchunk * 512,
                                        half * IH + (nchunk + 1) * 512)
                            nc.tensor.matmul(
                                sc[:, csl],
                                lhsT=kt_sb[t][prow:prow + DH,
                                              jb * P:(jb + 1) * P],
                                rhs=qt_sb[t][prow:prow + DH, qsl],
                                start=True, stop=True,
                            )
                        scs.append(sc)
                    for par in range(2):
                        et = expp.tile([P, IH], BF16, tag="et",
                                       name=f"et{par}")
                        nc.scalar.activation(
                            out=et, in_=scs[par],
                            func=mybir.ActivationFunctionType.Exp,
                            scale=0.125,
                        )
                        ets[par][jb] = et
                return ets

            def emit_av_norm(half, hp, ets):
                avs = [avpool.tile([DH + 1, IH], F32, tag="av",
                                   name=f"av{par}") for par in range(2)]
                for jb in range(JBN):
                    for par in range(2):
                        for nchunk in range(2):
                            csl = slice(nchunk * 512, (nchunk + 1) * 512)
                            nc.tensor.matmul(
                                avs[par][:, csl],
                                lhsT=v_sb[jb][:, 2 * hp + par, :],
                                rhs=ets[par][jb][:, csl],
                                start=(jb == 0), stop=(jb == JBN - 1),
                            )
                for par in range(2):
                    h = 2 * hp + par
                    av = avs[par]
                    # normalise: out2T = av[0:64] / av[64]
                    araw = misc.tile([DH + 1, IH], F32, tag="araw",
                                     name="araw")
                    nc.vector.tensor_copy(araw, av)
                    bidx = half * HPC + h
                    # reciprocal is ~7 cyc/elem/lane: transpose the denom
                    # row to [128, 8] via DRAM so all lanes share the work,
                    # then broadcast the recip row back across 64 partitions.
                    # All on the (idle during attention) gpsimd queue.
                    nc.gpsimd.dma_start(out=brc[bidx:bidx + 1, :],
                                        in_=araw[DH:DH + 1, :])
                    rcol = misc.tile([P, IH // P], F32, tag="rcol",
                                     name="rcol")
                    nc.gpsimd.dma_start(
                        out=rcol,
                        in_=brc[bidx, :].rearrange("(p t) -> p t", p=P),
                    )
                    rrec = misc.tile([P, IH // P], F32, tag="rrec",
                                     name="rrec")
                    nc.vector.reciprocal(rrec, rcol)
                    nc.gpsimd.dma_start(
                        out=brc2[bidx, :].rearrange("(p t) -> p t", p=P),
                        in_=rrec,
                    )
                    bc = misc.tile([DH, IH], F32, tag="bc", name="bc")
                    row = brc2[bidx:bidx + 1, :]
                    nc.gpsimd.dma_start(
                        out=bc,
                        in_=bass.AP(tensor=row.tensor, offset=row.offset,
                                    ap=[[0, DH]] + row.ap[1:]),
                    )
                    nc.vector.tensor_mul(
                        o2t_sb[half][hp][par * DH:par * DH + DH, :],
                        araw[0:DH, :], bc
                    )

            def emit_wo(half):
                # runs in the tail: both psum pools are free (extra slot
                # depth keeps the PE streaming), ScalarE is idle (evacs
                # alternate ACT/DVE), output staged bf16
                for m in range(IH // P):
                    ot = outp.tile([P, E], BF16, tag="ot", name="ot")
                    big = pp.tile([P, IH], F32, tag="pp", name="wobig")
                    pss = [big[:, 0:512], big[:, 512:1024]]
                    for t in range(TD):
                        for nchunk in range(2):
                            nc.tensor.matmul(
                                pss[nchunk],
                                lhsT=o2t_sb[half][t][:, m * P:(m + 1) * P],
                                rhs=wo_sb[:, t, nchunk * 512:(nchunk + 1) * 512],
                                start=(t == 0), stop=(t == TD - 1),
                            )
                    for nchunk in range(2):
                        dst = ot[:, nchunk * 512:(nchunk + 1) * 512]
                        if nchunk == 1:
                            nc.scalar.activation(
                                out=dst, in_=pss[nchunk],
                                func=mybir.ActivationFunctionType.Copy)
                        else:
                            nc.vector.tensor_copy(dst, pss[nchunk])
                    r0 = half * IH + m * P
                    nc.sync.dma_start(out=out[r0:r0 + P, :], in_=ot)

            pending = None
            for k, (half, hp) in enumerate([(0, 0), (0, 1), (1, 0), (1, 1)]):
                ets = emit_scores_exp(half, hp)
                if pending is not None:
                    emit_av_norm(*pending)
                if k == 0:
                    emit_qt(0, 1)
                    emit_qt(1, 1)
                    emit_qt(2, 0)
                    emit_qt(3, 0)
                elif k == 1:
                    emit_qt(2, 1)
                    emit_qt(3, 1)
                pending = (half, hp, ets)
            emit_av_norm(*pending)
            emit_wo(0)
            emit_wo(1)

    nc.compile()
    return nc


_NC_CACHE = None


def _get_nc():
    global _NC_CACHE
    if _NC_CACHE is None:
        _NC_CACHE = _build()
    return _NC_CACHE


def _sbuf_image(a):
    """[KB*128, R] row-major -> [128, KB*R]: partition p holds the
    concatenation of rows {kb*128+p} -- one contiguous run per partition."""
    kb = a.shape[0] // P
    return np.ascontiguousarray(
        a.reshape(kb, P, a.shape[1]).transpose(1, 0, 2).reshape(P, -1)
    ).astype(ml_dtypes.bfloat16)


def _make_in_maps(x, context, Wq, Wk, Wv, Wo):
    in_maps = []
    for c in range(N_CORES):
        b, hg = c // 2, c % 2
        sl = slice(hg * DG, (hg + 1) * DG)
        in_maps.append({
            "xt": _sbuf_image(x[b].T),
            "ctxt": _sbuf_image(context[b].T),
            "wq": _sbuf_image(Wq[:, sl]),
            "wk": _sbuf_image(Wk[:, sl]),
            "wv": _sbuf_image(Wv[:, sl]),
            "wo": _sbuf_image(Wo[sl, :]),
            "vones": np.ones((P, HPC), dtype=ml_dtypes.bfloat16),
        })
    return in_maps


def _run(inputs, trace=False):
    x = np.asarray(inputs["x"], dtype=np.float32)
    context = np.asarray(inputs["context"], dtype=np.float32)
    Wq = np.asarray(inputs["Wq"], dtype=np.float32)
    Wk = np.asarray(inputs["Wk"], dtype=np.float32)
    Wv = np.asarray(inputs["Wv"], dtype=np.float32)
    Wo = np.asarray(inputs["Wo"], dtype=np.float32)
    bo = np.asarray(inputs["bo"], dtype=np.float32)

    res = run_bass_kernel_spmd(
        _get_nc(), _make_in_maps(x, context, Wq, Wk, Wv, Wo),
        core_ids=list(range(N_CORES)), trace=trace,
    )
    parts = [np.asarray(r["out"], dtype=np.float32) for r in res.results]
    outv = np.stack([parts[2 * b] + parts[2 * b + 1] + bo for b in range(B)])
    return outv, res


def kernel(**inputs) -> np.ndarray:
    outv, _ = _run(inputs, trace=False)
    return outv
